# revision 1
# baseline (speedup 1.0000x reference)
"""Trainium2 Bass kernel v2 for nn_GaussianModel (gaussian splat into 256^3).

Math (per gaussian, separable):
    vol[x,y,z] += I * exp(-(jx^2+jy^2+jz^2)),  j* = sp*(coord - center)
with sp = 1/(sqrt(2)*255*sigma). The hard window mask of the reference is
DROPPED (tails beyond 3 sigma contribute ~0.6% L2, within tolerance) which
removes all mask ops. exp(-u^2) is computed with the fused Derivative_Erf
activation (= 2/sqrt(pi) * exp(-u^2)); the constant^3 is folded into I.

Decomposition: output tile = (x-bin 16) x (y-half 128) x (z-half 128);
vol[y,(x,z)] = Wy^T @ Wxz per tile, accumulating over 128-gaussian blocks in
PSUM [128, 2048] (4 banks). Everything bf16 except PSUM (f32) and the
per-gaussian params.

Sharding: output x-axis split into 8 slabs of 32 (one core each); per core 8
tiles ("slots"). Slots are sorted by size and PAIRED (0,7),(1,6),(2,5),(3,4);
each pair shares a row-packed block sequence (boundary 32-aligned, static
across cores via rank-max padding) so both PSUM groups accumulate from mixed
128-row blocks -> fewer blocks than per-slot ceil padding.

Per block: ACT Derivative_Erf computes wz,wy [128,128] (bf16) and wx
[128,16] (f32) directly from a shared iota tile with per-partition
scale/bias; 16 Kron ops wxz[:,l*128:+128] = wz * wx[:,l] * I (two-scalar
tensor_scalar) split across DVE/Pool; then 4 matmuls (N=512, bf16) per PSUM
group. A pair's boundary block uses two zero-masked wy copies so both
groups run full-K matmuls. PSUM is evacuated bf16 via one ACT + one DVE
copy per slot and DMAed; the host converts/unshuffles to the f32 volume.
"""

import sys
import numpy as np

for _p in ("/opt/trn_rl_repo", "/root/.axon_site/_ro/trn_rl_repo"):
    if _p not in sys.path:
        sys.path.append(_p)

SHAPE = (256, 256, 256)
N_CORES = 8
SLAB = 32                 # x-planes per core
XBIN = 16                 # x-planes per output tile
W = 26                    # reference's fixed window size
NPRM = 8                  # params per gaussian (padded to 8 for alignment)
NSLOT = 8                 # tiles per core: 2 xbins x 2 yhalves x 2 zhalves
PAIRS = ((0, 7), (1, 6), (2, 5), (3, 4))   # slot-rank pairing
PAIR_MIXED = True   # allow a 64-row A/B split inside one block
STAGGERED = False   # For_i staggered_reset (bench loops only)

C_BY, C_BZ, C_BX, C_SP, C_I = range(5)

# engine split of the 16 kron ops per block (tuned via TimelineSim)
KRON_DVE = 11
KRON_POOL = 5   # remaining 16 - KRON_DVE - KRON_POOL go to ACT
# evac split of [128, 2048] per slot: (ACT, DVE) column widths
# (GPSIMD cannot read PSUM, so evacuation is ACT+DVE only)
EVAC_SPLIT = (1280, 768)


def _host_pack(centers, sigmas, intensities):
    """Replicate the reference's f32 window math; bucket/pair/pack per core."""
    f32 = np.float32
    c = np.asarray(centers, f32)
    sg = np.asarray(sigmas, f32)
    it = np.asarray(intensities, f32)
    n = c.shape[0]

    scale = f32(255.0)
    cv = c * scale
    cut = (f32(3.0) * sg)[:, None] * np.full((3,), scale, f32)[None, :]
    min_i = np.maximum(cv - cut, f32(0.0)).astype(np.int32)
    max_i = np.minimum((np.minimum(cv + cut, scale) + f32(1.0)).astype(np.int32), 256)
    hi = np.minimum(max_i, min_i + W).astype(f32)
    lo = min_i.astype(f32)

    # assignment window: slightly tighter than the true +-3 sigma window.
    # A gaussian is only assigned to tiles its +-2.7 sigma box touches; the
    # skipped outer shell carries <= e^-3.65 ~ 2.6% of peak and only in
    # neighbouring tiles, adding ~1e-3 L2 — well inside tolerance.
    acut = (f32(2.7) * sg)[:, None] * np.full((3,), scale, f32)[None, :]
    alo = np.maximum(np.maximum(cv - acut, f32(0.0)).astype(np.int32).astype(f32), lo)
    ahi = np.minimum(
        np.minimum((np.minimum(cv + acut, scale) + f32(1.0)).astype(np.int32), 256)
        .astype(f32), hi)

    active_cnt = int((sg > 0).sum())
    keep = (np.arange(n) < active_cnt) & (sg > 0)

    sp = np.zeros(n, f32)
    sp[keep] = f32(1.0) / (f32(np.sqrt(2.0)) * scale * sg[keep])
    # Derivative_Erf(u) = 2/sqrt(pi) exp(-u^2) applied once per axis
    ip = (it * f32((np.sqrt(np.pi) / 2.0) ** 3)).astype(f32)
    gidx = np.nonzero(keep)[0]

    # buckets[core] = list of (meta=(xb,hy,hz), gaussian idx array), sorted desc
    buckets = []
    for ci in range(N_CORES):
        bl = []
        for xb in range(SLAB // XBIN):
            xs = SLAB * ci + XBIN * xb
            inx = gidx[(alo[gidx, 0] < xs + XBIN) & (ahi[gidx, 0] > xs)]
            for hy in range(2):
                ys = 128 * hy
                iny = inx[(alo[inx, 1] < ys + 128) & (ahi[inx, 1] > ys)]
                for hz in range(2):
                    zs = 128 * hz
                    g = iny[(alo[iny, 2] < zs + 128) & (ahi[iny, 2] > zs)]
                    bl.append(((xb, hy, hz), g))
        bl.sort(key=lambda t: -len(t[1]))
        buckets.append(bl)

    # rank-max unit counts (32-row units) per slot rank; the A/B boundary
    # inside a mixed block must land on partition 0 or 64 (matmul base
    # partition constraint), so uA is rounded to an even unit count.
    units = np.array([[(len(buckets[i][k][1]) + 31) // 32 for k in range(NSLOT)]
                      for i in range(N_CORES)])
    u_rank = units.max(axis=0)          # [NSLOT]
    pair_meta = []                      # (rankA, rankB, uA, uB, nblk)
    for ra, rb in PAIRS:
        uA, uB = int(u_rank[ra]), int(u_rank[rb])
        uA += uA % 2
        if not PAIR_MIXED:              # block-align the A/B boundary
            uA = ((uA + 3) // 4) * 4
        nblk = (uA + uB + 3) // 4
        pair_meta.append((ra, rb, uA, uB, nblk))
    nbtot = sum(m[4] for m in pair_meta)

    payloads = []
    for i in range(N_CORES):
        prm = np.zeros((max(nbtot, 1), 128, NPRM), f32)
        slotmap = [None] * NSLOT
        base = 0
        for (ra, rb, uA, uB, nblk) in pair_meta:
            rows = prm[base:base + nblk].reshape(-1, NPRM)
            for rank, u0, in ((ra, 0), (rb, uA)):
                (xb, hy, hz), g = buckets[i][rank]
                slotmap[rank] = (xb, hy, hz)
                kk = len(g)
                r0 = 32 * u0
                xs = f32(SLAB * i + XBIN * xb)
                ys, zs = f32(128.0 * hy), f32(128.0 * hz)
                rows[r0:r0 + kk, C_BY] = sp[g] * (ys - cv[g, 1])
                rows[r0:r0 + kk, C_BZ] = sp[g] * (zs - cv[g, 2])
                rows[r0:r0 + kk, C_BX] = sp[g] * (xs - cv[g, 0])
                rows[r0:r0 + kk, C_SP] = sp[g]
                rows[r0:r0 + kk, C_I] = ip[g]
            base += nblk
        payloads.append({"prm": prm, "slotmap": slotmap})

    return payloads, pair_meta


def _build_kernel(pair_meta, reps=1, loop_reps=None):
    """Build + compile the 8-core SPMD Bass program.

    reps>1 unrolls the whole compute (identical work/results) for benching;
    loop_reps additionally wraps the unrolled body in a hardware For_i loop,
    giving reps*loop_reps total repetitions with constant program size:
    steady-state HW time = (t(R_big) - t(R_small)) / (R_big - R_small).
    """
    from concourse import bacc, tile
    import concourse.mybir as mybir

    f32 = mybir.dt.float32
    bf16 = mybir.dt.bfloat16
    AF = mybir.ActivationFunctionType
    OP = mybir.AluOpType

    nbtot = max(sum(m[4] for m in pair_meta), 1)

    nc = bacc.Bacc("TRN2", target_bir_lowering=False, debug=False,
                   num_devices=N_CORES)
    io_t = nc.dram_tensor("iota", (128, 128), f32, kind="ExternalInput")
    prm_t = nc.dram_tensor("prm", (nbtot, 128, NPRM), f32, kind="ExternalInput")
    vol_t = nc.dram_tensor("vol", (NSLOT, 128, XBIN * 128), bf16,
                           kind="ExternalOutput")

    with tile.TileContext(nc) as tc:
        with (
            tc.tile_pool(name="const", bufs=1) as cpool,
            tc.tile_pool(name="work", bufs=4) as wpool,
            tc.tile_pool(name="kron", bufs=4) as kpool,
            tc.tile_pool(name="evac", bufs=3) as opool,
            tc.tile_pool(name="psum", bufs=1, space="PSUM") as ppool,
        ):
            io32 = cpool.tile([128, 128], f32)
            nc.sync.dma_start(io32[:], io_t.ap())
            prm_sb = cpool.tile([128, nbtot * NPRM], f32)
            for blk in range(nbtot):
                nc.sync.dma_start(
                    prm_sb[:, blk * NPRM:(blk + 1) * NPRM], prm_t.ap()[blk])
            # touch Derivative_Erf once before the rep loop so the act-table
            # load is hoisted out of the For_i body
            warm = cpool.tile([128, 1], bf16)
            nc.scalar.activation(warm[:], io32[:, 0:1], AF.Derivative_Erf)

            def emit_block(blk, split=False):
                """DErf axis weights -> kron. Returns (wys, wxz) where wys is
                [wyA, wyB] for a 64-row mixed block (upper/lower halves
                zero-masked so both groups run full-K matmuls) else [wy]."""
                P = lambda col: prm_sb[:, blk * NPRM + col: blk * NPRM + col + 1]
                wyz = wpool.tile([128, 256], bf16, tag="wyz")
                # z then x first (kron inputs), y last (only matmul needs it)
                nc.scalar.activation(wyz[:, 128:256], io32[:], AF.Derivative_Erf,
                                     bias=P(C_BZ), scale=P(C_SP))
                wx = wpool.tile([128, XBIN], f32, tag="wx")
                nc.scalar.activation(wx[:], io32[:, 0:XBIN], AF.Derivative_Erf,
                                     bias=P(C_BX), scale=P(C_SP))
                if split:
                    wyA = wpool.tile([128, 128], bf16, tag="wyA")
                    nc.scalar.activation(wyA[0:64, :], io32[0:64, :],
                                         AF.Derivative_Erf,
                                         bias=P(C_BY)[0:64], scale=P(C_SP)[0:64])
                    nc.gpsimd.memset(wyA[64:128, :], 0.0)
                    wyB = wpool.tile([128, 128], bf16, tag="wyB")
                    nc.scalar.activation(wyB[64:128, :], io32[64:128, :],
                                         AF.Derivative_Erf,
                                         bias=P(C_BY)[64:128],
                                         scale=P(C_SP)[64:128])
                    nc.gpsimd.memset(wyB[0:64, :], 0.0)
                    wys = [wyA[:], wyB[:]]
                else:
                    nc.scalar.activation(wyz[:, 0:128], io32[:],
                                         AF.Derivative_Erf,
                                         bias=P(C_BY), scale=P(C_SP))
                    wys = [wyz[:, 0:128]]

                wxz = kpool.tile([128, XBIN * 128], bf16, tag="wxz")
                wz = wyz[:, 128:256]
                for xl in range(XBIN):
                    dst = wxz[:, xl * 128:(xl + 1) * 128]
                    sc = wx[:, xl:xl + 1]
                    if xl < KRON_DVE:
                        nc.vector.tensor_scalar(dst, wz, sc, P(C_I),
                                                op0=OP.mult, op1=OP.mult)
                    else:
                        nc.gpsimd.tensor_scalar(dst, wz, sc, P(C_I),
                                                op0=OP.mult, op1=OP.mult)
                return wys, wxz

            def evac_slot(ps, rank):
                st = opool.tile([128, XBIN * 128], bf16, tag="st")
                a, d = EVAC_SPLIT
                nc.scalar.activation(st[:, 0:a], ps[:, 0:a], AF.Copy)
                nc.vector.tensor_copy(st[:, a:a + d], ps[:, a:a + d])
                nc.sync.dma_start(vol_t.ap()[rank], st[:])

            def emit_rep():
                base = 0
                for (ra, rb, uA, uB, nblk) in pair_meta:
                    if nblk == 0 or (uA == 0 and uB == 0):
                        continue
                    psA = psB = None
                    if uA:
                        psA = ppool.tile([128, XBIN * 128], f32,
                                         name="psA", tag="psA")
                    if uB:
                        psB = ppool.tile([128, XBIN * 128], f32,
                                         name="psB", tag="psB")
                    # block index ranges per group (in units of 32 rows)
                    lastA = (uA - 1) // 4 if uA else -1
                    firstB = uA // 4 if uB else nblk
                    for b in range(nblk):
                        blk = base + b
                        mixed = (uA % 4 == 2) and b == lastA == firstB
                        wys, wxz = emit_block(blk, split=mixed)
                        groups = []   # (psum, wy AP, start, stop)
                        if uA and b <= lastA:
                            groups.append((psA, wys[0], b == 0, b == lastA))
                        if uB and b >= firstB:
                            groups.append((psB, wys[-1], b == firstB,
                                           b == nblk - 1))
                        for ps, wy_ap, st_flag, sp_flag in groups:
                            for nn in range(4):
                                nc.tensor.matmul(
                                    ps[:, nn * 512:(nn + 1) * 512],
                                    wy_ap,
                                    wxz[:, nn * 512:(nn + 1) * 512],
                                    start=st_flag, stop=sp_flag)
                        if uA and b == lastA:
                            evac_slot(psA, ra)
                    if uB:
                        evac_slot(psB, rb)
                    base += nblk

            if loop_reps is not None:
                ET = mybir.EngineType
                with tc.For_i(0, loop_reps, 1,
                              hint_engines=(ET.PE, ET.DVE, ET.Activation,
                                            ET.Pool, ET.SP),
                              staggered_reset=STAGGERED):
                    for _ in range(reps):
                        emit_rep()
            else:
                for _ in range(reps):
                    emit_rep()

    nc.compile()
    return nc


def _make_inputs(payloads):
    iota_np = np.broadcast_to(
        np.arange(128, dtype=np.float32), (128, 128)).copy()
    return [{"iota": iota_np, "prm": p["prm"]} for p in payloads]


def _assemble(results, payloads):
    out = np.empty(SHAPE, np.float32)
    for i in range(N_CORES):
        v = np.asarray(results[i]["vol"]).astype(np.float32)  # [8,128,2048]
        for k, (xb, hy, hz) in enumerate(payloads[i]["slotmap"]):
            q = v[k].reshape(128, XBIN, 128).transpose(1, 0, 2)
            out[SLAB * i + XBIN * xb: SLAB * i + XBIN * (xb + 1),
                128 * hy:128 * (hy + 1),
                128 * hz:128 * (hz + 1)] = q
    return out


def _run(inputs, trace=False):
    from concourse import bass_utils

    payloads, pair_meta = _host_pack(
        inputs["centers"], inputs["sigmas"], inputs["intensities"])
    nc = _build_kernel(pair_meta)
    res = bass_utils.run_bass_kernel_spmd(
        nc, _make_inputs(payloads), core_ids=list(range(N_CORES)), trace=trace)
    return _assemble(res.results, payloads), res


def kernel(centers, sigmas, intensities):
    out, _ = _run({"centers": centers, "sigmas": sigmas,
                   "intensities": intensities})
    return out


if __name__ == "__main__":
    rng = np.random.default_rng(0)
    c = rng.random((100, 3), np.float32)
    s = (0.004 + 0.011 * rng.random(100)).astype(np.float32)
    i = rng.random(100, np.float32)
    v = kernel(centers=c, sigmas=s, intensities=i)
    print(v.shape, v.dtype, v.sum())



# revision 5
# speedup vs baseline: 1.0386x; 1.0386x over previous
"""Trainium2 Bass kernel v3 for nn_GaussianModel (gaussian splat into 256^3).

Math (per gaussian, separable):
    vol[x,y,z] += I * exp(-(jx^2+jy^2+jz^2)),  j* = sp*(coord - center)
with sp = 1/(sqrt(2)*255*sigma). The hard window mask of the reference is
DROPPED (tails beyond 3 sigma contribute ~0.6% L2, within tolerance);
exp(-u^2) is computed with the fused Derivative_Erf activation
(= 2/sqrt(pi) * exp(-u^2)); the constant^3 is folded into I.

Decomposition: output tile ("slot") = (x-bin 16) x (y-half 128) x (z-half
128); vol[y,(x,z)] = Wy^T @ Wxz per slot, accumulating 128-gaussian blocks in
PSUM [128, 2048] f32 (4 banks). Everything bf16 except PSUM and params.

Sharding: output x split into 8 slabs of 32 (one core each); 8 slots per
core. v3 packs all 8 slots SEQUENTIALLY into one block stream at 64-row
granularity (rank-maxed across cores so the SPMD program is shared): a block
holds up to two slot segments (boundary at row 64). Each segment's wy is a
full-128-row DErf whose params are host-masked (scale=0, bias=30 -> DErf~=0)
outside the segment, so no memsets or partial-partition ops are needed, and
both segments' matmuls run full-K. PSUM tiles rotate through 2 buffers
(4 banks each), so slot k's evac overlaps slot k+1's matmuls.

Per block: ACT computes wz [128,128](bf16) / wx [128,16](f32) / one wy per
segment from a shared iota with per-partition scale/bias; 16 kron ops
wxz[:, l*128:+128] = wz * wx[:,l] * I (two-scalar tensor_scalar) split
DVE/POOL/ACT; 4 matmuls (N=512) per segment. Evac: one whole-slot PSUM->SBUF
bf16 copy on ACT or DVE (per-slot pattern) + DMA; host unshuffles to f32.
"""

import sys
import numpy as np

for _p in ("/opt/trn_rl_repo", "/root/.axon_site/_ro/trn_rl_repo"):
    if _p not in sys.path:
        sys.path.append(_p)

SHAPE = (256, 256, 256)
N_CORES = 8
SLAB = 32                 # x-planes per core
XBIN = 16                 # x-planes per output tile
W = 26                    # reference's fixed window size
NPRM = 8                  # params per gaussian
NSLOT = 8                 # tiles per core: 2 xbins x 2 yhalves x 2 zhalves

C_BZ, C_BX, C_SP, C_I, C_BY0, C_SP0, C_BY1, C_SP1 = range(8)

# engine split of the 16 kron ops per block (DVE, POOL; rest go to ACT)
KRON_DVE = 11
KRON_POOL = 5
# evac engine per slot rank: 'A' (ACT) or 'D' (DVE)
EVAC_ENGINES = "ADAADADA"


def _host_pack(centers, sigmas, intensities):
    """Replicate the reference's f32 window math; bucket/pack per core.

    Returns (payloads, blocks) where blocks is the static shared block
    structure: blocks[b] = list of segments (rank, seg_idx, first, last).
    """
    f32 = np.float32
    c = np.asarray(centers, f32)
    sg = np.asarray(sigmas, f32)
    it = np.asarray(intensities, f32)
    n = c.shape[0]

    scale = f32(255.0)
    cv = c * scale
    cut = (f32(3.0) * sg)[:, None] * np.full((3,), scale, f32)[None, :]
    min_i = np.maximum(cv - cut, f32(0.0)).astype(np.int32)
    max_i = np.minimum((np.minimum(cv + cut, scale) + f32(1.0)).astype(np.int32), 256)
    hi = np.minimum(max_i, min_i + W).astype(f32)
    lo = min_i.astype(f32)

    # assignment window: slightly tighter than the true +-3 sigma window.
    # A gaussian is only assigned to tiles its +-2.7 sigma box touches; the
    # skipped outer shell carries <= e^-3.65 ~ 2.6% of peak and only in
    # neighbouring tiles, adding ~1e-3 L2 — well inside tolerance.
    acut = (f32(2.7) * sg)[:, None] * np.full((3,), scale, f32)[None, :]
    alo = np.maximum(np.maximum(cv - acut, f32(0.0)).astype(np.int32).astype(f32), lo)
    ahi = np.minimum(
        np.minimum((np.minimum(cv + acut, scale) + f32(1.0)).astype(np.int32), 256)
        .astype(f32), hi)

    active_cnt = int((sg > 0).sum())
    keep = (np.arange(n) < active_cnt) & (sg > 0)

    sp = np.zeros(n, f32)
    sp[keep] = f32(1.0) / (f32(np.sqrt(2.0)) * scale * sg[keep])
    # Derivative_Erf(u) = 2/sqrt(pi) exp(-u^2) applied once per axis
    ip = (it * f32((np.sqrt(np.pi) / 2.0) ** 3)).astype(f32)
    gidx = np.nonzero(keep)[0]

    # buckets[core] = list of (meta=(xb,hy,hz), gaussian idx array), sorted desc
    buckets = []
    for ci in range(N_CORES):
        bl = []
        for xb in range(SLAB // XBIN):
            xs = SLAB * ci + XBIN * xb
            inx = gidx[(alo[gidx, 0] < xs + XBIN) & (ahi[gidx, 0] > xs)]
            for hy in range(2):
                ys = 128 * hy
                iny = inx[(alo[inx, 1] < ys + 128) & (ahi[inx, 1] > ys)]
                for hz in range(2):
                    zs = 128 * hz
                    g = iny[(alo[iny, 2] < zs + 128) & (ahi[iny, 2] > zs)]
                    bl.append(((xb, hy, hz), g))
        bl.sort(key=lambda t: -len(t[1]))
        buckets.append(bl)

    # rank-max 64-row unit counts per slot rank (shared SPMD structure)
    u64 = np.array([[(len(buckets[i][k][1]) + 63) // 64 for k in range(NSLOT)]
                    for i in range(N_CORES)])
    u_rank = u64.max(axis=0)            # [NSLOT]
    starts = np.concatenate([[0], np.cumsum(u_rank)])
    tot_units = int(starts[-1])
    nblk = (tot_units + 1) // 2

    # static block structure: blocks[b] = [(rank, seg_idx, first, last), ...]
    blocks = [[] for _ in range(nblk)]
    for k in range(NSLOT):
        s, e = int(starts[k]), int(starts[k + 1])
        if s == e:
            continue
        for b in range(s // 2, (e + 1) // 2):
            u0, u1 = max(s, 2 * b), min(e, 2 * b + 2)
            seg_idx = 0 if u0 == 2 * b else 1
            blocks[b].append((k, seg_idx, u0 == s, u1 == e))

    payloads = []
    for i in range(N_CORES):
        prm = np.zeros((max(nblk, 1), 128, NPRM), f32)
        prm[:, :, C_BY0] = 30.0
        prm[:, :, C_BY1] = 30.0
        slotmap = [None] * NSLOT
        rows = prm.reshape(-1, NPRM)
        for k in range(NSLOT):
            (xb, hy, hz), g = buckets[i][k]
            slotmap[k] = (xb, hy, hz)
            kk = len(g)
            r0 = 64 * int(starts[k])
            xs = f32(SLAB * i + XBIN * xb)
            ys, zs = f32(128.0 * hy), f32(128.0 * hz)
            rows[r0:r0 + kk, C_BZ] = sp[g] * (zs - cv[g, 2])
            rows[r0:r0 + kk, C_BX] = sp[g] * (xs - cv[g, 0])
            rows[r0:r0 + kk, C_SP] = sp[g]
            rows[r0:r0 + kk, C_I] = ip[g]
            # per-segment masked wy params: a row's wy param set is 0 if the
            # slot's coverage of that row's block starts at block row 0
            # (equivalently at an even unit), else 1.
            for j in range(kk):
                r = r0 + j
                u0 = max(2 * (r // 128), int(starts[k]))
                by, spc = (C_BY0, C_SP0) if u0 % 2 == 0 else (C_BY1, C_SP1)
                rows[r, by] = sp[g[j]] * (ys - cv[g[j], 1])
                rows[r, spc] = sp[g[j]]
        payloads.append({"prm": prm, "slotmap": slotmap})

    return payloads, blocks


def _build_kernel(blocks, reps=1, loop_reps=None):
    """Build + compile the 8-core SPMD Bass program.

    reps>1 unrolls the whole compute (identical work/results) for benching;
    loop_reps additionally wraps the unrolled body in a hardware For_i loop.
    """
    from concourse import bacc, tile
    import concourse.mybir as mybir

    f32 = mybir.dt.float32
    bf16 = mybir.dt.bfloat16
    AF = mybir.ActivationFunctionType
    OP = mybir.AluOpType

    nblk = max(len(blocks), 1)

    nc = bacc.Bacc("TRN2", target_bir_lowering=False, debug=False,
                   num_devices=N_CORES)
    io_t = nc.dram_tensor("iota", (128, 128), f32, kind="ExternalInput")
    prm_t = nc.dram_tensor("prm", (nblk, 128, NPRM), f32, kind="ExternalInput")
    vol_t = nc.dram_tensor("vol", (NSLOT, 128, XBIN * 128), bf16,
                           kind="ExternalOutput")

    with tile.TileContext(nc) as tc:
        with (
            tc.tile_pool(name="const", bufs=1) as cpool,
            tc.tile_pool(name="work", bufs=4) as wpool,
            tc.tile_pool(name="kron", bufs=4) as kpool,
            tc.tile_pool(name="evac", bufs=3) as opool,
            tc.tile_pool(name="psum", bufs=2, space="PSUM") as ppool,
        ):
            io32 = cpool.tile([128, 128], f32)
            nc.sync.dma_start(io32[:], io_t.ap())
            prm_sb = cpool.tile([128, nblk * NPRM], f32)
            for blk in range(nblk):
                nc.sync.dma_start(
                    prm_sb[:, blk * NPRM:(blk + 1) * NPRM], prm_t.ap()[blk])
            # touch Derivative_Erf once before the rep loop so the act-table
            # load is hoisted out of the For_i body
            warm = cpool.tile([128, 1], bf16)
            nc.scalar.activation(warm[:], io32[:, 0:1], AF.Derivative_Erf)

            def emit_block(blk):
                """DErf wz/wx -> kron wxz. Returns wxz tile."""
                P = lambda col: prm_sb[:, blk * NPRM + col: blk * NPRM + col + 1]
                wz = wpool.tile([128, 128], bf16, tag="wz")
                nc.scalar.activation(wz[:], io32[:], AF.Derivative_Erf,
                                     bias=P(C_BZ), scale=P(C_SP))
                wx = wpool.tile([128, XBIN], f32, tag="wx")
                nc.scalar.activation(wx[:], io32[:, 0:XBIN], AF.Derivative_Erf,
                                     bias=P(C_BX), scale=P(C_SP))
                wxi = None
                if KRON_DVE + KRON_POOL < XBIN:
                    # ACT kron cannot apply two scalars; fold I into wx once
                    wxi = wpool.tile([128, XBIN], f32, tag="wxi")
                    nc.vector.tensor_scalar(wxi[:], wx[:], P(C_I), None,
                                            op0=OP.mult)
                wxz = kpool.tile([128, XBIN * 128], bf16, tag="wxz")
                for xl in range(XBIN):
                    dst = wxz[:, xl * 128:(xl + 1) * 128]
                    sc = wx[:, xl:xl + 1]
                    if xl < KRON_DVE:
                        nc.vector.tensor_scalar(dst, wz[:], sc, P(C_I),
                                                op0=OP.mult, op1=OP.mult)
                    elif xl < KRON_DVE + KRON_POOL:
                        nc.gpsimd.tensor_scalar(dst, wz[:], sc, P(C_I),
                                                op0=OP.mult, op1=OP.mult)
                    else:
                        nc.scalar.activation(dst, wz[:], AF.Copy,
                                             scale=wxi[:, xl:xl + 1])
                return wxz

            def emit_wy(blk, seg):
                P = lambda col: prm_sb[:, blk * NPRM + col: blk * NPRM + col + 1]
                by, sp = (C_BY0, C_SP0) if seg == 0 else (C_BY1, C_SP1)
                wy = wpool.tile([128, 128], bf16, tag=f"wy{seg}")
                nc.scalar.activation(wy[:], io32[:], AF.Derivative_Erf,
                                     bias=P(by), scale=P(sp))
                return wy

            def evac_slot(ps, rank):
                st = opool.tile([128, XBIN * 128], bf16, tag="st")
                if EVAC_ENGINES[rank % len(EVAC_ENGINES)] == "A":
                    nc.scalar.activation(st[:], ps[:], AF.Copy)
                else:
                    nc.vector.tensor_copy(st[:], ps[:])
                nc.sync.dma_start(vol_t.ap()[rank], st[:])

            def emit_rep():
                ps_of = {}
                for b, segs in enumerate(blocks):
                    if not segs:
                        continue
                    wxz = emit_block(b)
                    for (rank, seg, first, last) in segs:
                        if first:
                            ps_of[rank] = ppool.tile([128, XBIN * 128], f32,
                                                     name=f"ps{rank}", tag="ps")
                        ps = ps_of[rank]
                        wy = emit_wy(b, seg)
                        for nn in range(4):
                            nc.tensor.matmul(
                                ps[:, nn * 512:(nn + 1) * 512],
                                wy[:],
                                wxz[:, nn * 512:(nn + 1) * 512],
                                start=first, stop=last)
                        if last:
                            evac_slot(ps, rank)
                            del ps_of[rank]

            if loop_reps is not None:
                ET = mybir.EngineType
                with tc.For_i(0, loop_reps, 1,
                              hint_engines=(ET.PE, ET.DVE, ET.Activation,
                                            ET.Pool, ET.SP)):
                    for _ in range(reps):
                        emit_rep()
            else:
                for _ in range(reps):
                    emit_rep()

    nc.compile()
    return nc


def _make_inputs(payloads):
    iota_np = np.broadcast_to(
        np.arange(128, dtype=np.float32), (128, 128)).copy()
    return [{"iota": iota_np, "prm": p["prm"]} for p in payloads]


def _assemble(results, payloads):
    out = np.empty(SHAPE, np.float32)
    for i in range(N_CORES):
        v = np.asarray(results[i]["vol"]).astype(np.float32)  # [8,128,2048]
        for k, (xb, hy, hz) in enumerate(payloads[i]["slotmap"]):
            q = v[k].reshape(128, XBIN, 128).transpose(1, 0, 2)
            out[SLAB * i + XBIN * xb: SLAB * i + XBIN * (xb + 1),
                128 * hy:128 * (hy + 1),
                128 * hz:128 * (hz + 1)] = q
    return out


def _run(inputs, trace=False):
    from concourse import bass_utils

    payloads, blocks = _host_pack(
        inputs["centers"], inputs["sigmas"], inputs["intensities"])
    nc = _build_kernel(blocks)
    res = bass_utils.run_bass_kernel_spmd(
        nc, _make_inputs(payloads), core_ids=list(range(N_CORES)), trace=trace)
    return _assemble(res.results, payloads), res


def kernel(centers, sigmas, intensities):
    out, _ = _run({"centers": centers, "sigmas": sigmas,
                   "intensities": intensities})
    return out


if __name__ == "__main__":
    rng = np.random.default_rng(0)
    c = rng.random((100, 3), np.float32)
    s = (0.004 + 0.011 * rng.random(100)).astype(np.float32)
    i = rng.random(100, np.float32)
    v = kernel(centers=c, sigmas=s, intensities=i)
    print(v.shape, v.dtype, v.sum())
